# revision 1
# baseline (speedup 1.0000x reference)
"""2-layer GCN encoder on 8 Trainium2 NeuronCores.

Strategy (graph/data parallel, per sharding hint):
  - Nodes are permuted into NCORES x BPC x 128 slots, degree-balanced so
    every destination block's incoming-edge count fits a fixed chunk budget.
    Each core owns BPC destination blocks.
  - GCN layer out = relu(dinv_d * (A @ (dinv_s * tab)) @ W) via linearity:
    gather *pre-transform* rows tab[src], reduce per dst-block with one-hot
    segment matmuls on TensorE (seg[i, j] = dinv[src_i] one-hot by dst_rel,
    host-built), then one 128x128 post-matmul with W per block. Self-loops
    are one extra diagonal seg chunk per block.
  - Layer 1 gathers from the replicated bf16 feature table; layer 2 gathers
    from the layer-1 output table, exchanged with an 8-core AllGather.
  - dma_gather indices are int16, so the slot space is split into lo/hi
    halves with separate gather streams.
"""

import sys
import numpy as np

for _p in ("/opt/trn_rl_repo", "/root/.axon_site/_ro/trn_rl_repo"):
    if _p not in sys.path:
        sys.path.append(_p)

import ml_dtypes

import concourse.bass as bass
from concourse import bacc, mybir, tile
from concourse import bass_utils
from concourse.masks import make_identity

bf16 = ml_dtypes.bfloat16
P = 128


class Cfg:
    def __init__(self, n, ncores=8, bpc=49, cpc=8):
        self.N = n
        self.NCORES = ncores
        self.BPC = bpc                      # dst blocks per core
        self.CPC = cpc                      # 128-msg chunks per dma_gather call
        assert cpc * P <= 1024              # HW limit: dma_gather crashes above 1024 idxs/call
        self.NB = ncores * bpc              # total blocks
        assert self.NB % 2 == 0
        self.NPAD = self.NB * P
        self.HALF = self.NPAD // 2          # slots per src half
        self.NHALF_NODES = n // 2
        self.SPC = bpc * P                  # slots per core
        assert self.NPAD >= n and self.HALF < 32768


CFG_FULL = Cfg(50000)
C = 128


def _pack_half(deg_lo, deg_hi, node_ids, nblocks):
    """Greedily assign node_ids to nblocks bins of 128 slots, balancing both
    lo and hi incoming-edge sums. Returns [nblocks, 128] node ids (-1 pad)."""
    dl = deg_lo[node_ids].astype(np.int64)
    dh = deg_hi[node_ids].astype(np.int64)
    order = np.argsort(-(dl + dh), kind="stable")
    bins_cnt = np.zeros(nblocks, np.int64)
    bins_lo = np.zeros(nblocks, np.int64)
    bins_hi = np.zeros(nblocks, np.int64)
    slots = np.full((nblocks, P), -1, np.int64)
    BIG = 1 << 40
    for i in order:
        score = np.maximum(bins_lo + dl[i], bins_hi + dh[i])
        score = score + (bins_cnt >= P) * BIG
        b = int(np.argmin(score))
        slots[b, bins_cnt[b]] = node_ids[i]
        bins_cnt[b] += 1
        bins_lo[b] += dl[i]
        bins_hi[b] += dh[i]
    return slots


def _preprocess(x, edge_index, cfg):
    n = cfg.N
    src = np.asarray(edge_index[0], dtype=np.int64)
    dst = np.asarray(edge_index[1], dtype=np.int64)
    deg = 1 + np.bincount(dst, minlength=n)
    dinv = (1.0 / np.sqrt(deg)).astype(np.float32)

    is_lo = src < cfg.NHALF_NODES
    deg_lo = np.bincount(dst[is_lo], minlength=n)
    deg_hi = np.bincount(dst[~is_lo], minlength=n)

    slots_lo = _pack_half(deg_lo, deg_hi, np.arange(0, cfg.NHALF_NODES), cfg.NB // 2)
    slots_hi = _pack_half(deg_lo, deg_hi, np.arange(cfg.NHALF_NODES, n), cfg.NB // 2)
    slot_to_node = np.concatenate([slots_lo.reshape(-1), slots_hi.reshape(-1)])
    node_to_slot = np.full(n, -1, np.int64)
    valid = slot_to_node >= 0
    node_to_slot[slot_to_node[valid]] = np.nonzero(valid)[0]
    assert (node_to_slot >= 0).all()

    dinv_slot = np.zeros(cfg.NPAD, np.float32)
    dinv_slot[valid] = dinv[slot_to_node[valid]]

    s_slot = node_to_slot[src]
    d_slot = node_to_slot[dst]
    gb = d_slot >> 7
    jcol = d_slot & 127
    e_is_lo = s_slot < cfg.HALF

    def grouped_pad(mask, a_chunks, idx_off):
        gbm = gb[mask]
        ssm = s_slot[mask]
        jm = jcol[mask]
        cnt = np.bincount(gbm, minlength=cfg.NB)
        cap = a_chunks * P
        assert cnt.max() <= cap, (cnt.max(), cap)
        order = np.argsort(gbm, kind="stable")
        starts = np.zeros(cfg.NB, np.int64)
        starts[1:] = np.cumsum(cnt)[:-1]
        pos = np.arange(len(gbm)) - np.repeat(starts, cnt)
        idx_pad = np.zeros((cfg.NB, cap), np.int16)
        j_pad = np.full((cfg.NB, cap), -1, np.int32)
        v_pad = np.zeros((cfg.NB, cap), np.float32)
        gbs = gbm[order]
        idx_pad[gbs, pos] = (ssm[order] - idx_off).astype(np.int16)
        j_pad[gbs, pos] = jm[order]
        v_pad[gbs, pos] = dinv_slot[ssm[order]]
        return idx_pad, j_pad, v_pad

    a_lo = int(-(-np.bincount(gb[e_is_lo], minlength=cfg.NB).max() // P))
    a_hi = int(-(-np.bincount(gb[~e_is_lo], minlength=cfg.NB).max() // P))
    idx_lo, j_lo, v_lo = grouped_pad(e_is_lo, a_lo, 0)
    idx_hi, j_hi, v_hi = grouped_pad(~e_is_lo, a_hi, cfg.HALF)
    cpb = a_lo + a_hi + 1

    x = np.asarray(x, dtype=np.float32)
    x_tab = np.zeros((cfg.NPAD, C), bf16)
    x_tab[valid] = x[slot_to_node[valid]].astype(bf16)

    def wrap_calls(arr_flat, call_len):
        """Wrap a flat idx stream into the [128, cols] SBUF layout, 16-wrapped
        per dma_gather call of `call_len` idxs (short final call allowed)."""
        parts = []
        for s in range(0, arr_flat.size, call_len):
            a = arr_flat[s:s + call_len]
            parts.append(a.reshape(-1, 16).T)
        a = np.concatenate(parts, axis=1)
        return np.tile(a, (8, 1)).astype(np.int16)

    per_core = []
    for c in range(cfg.NCORES):
        blocks = np.arange(cfg.BPC) + cfg.BPC * c
        seg = np.zeros((cfg.BPC, cpb * P, P), np.float32)
        for jp, vp, off in ((j_lo, v_lo, 0), (j_hi, v_hi, a_lo * P)):
            jb = jp[blocks]
            bpos, ppos = np.nonzero(jb >= 0)
            seg[bpos, off + ppos, jb[bpos, ppos]] = vp[blocks][bpos, ppos]
        di_blk = dinv_slot[c * cfg.SPC:(c + 1) * cfg.SPC].reshape(cfg.BPC, P)
        ii = np.arange(P)
        seg[:, (a_lo + a_hi) * P + ii, ii] = di_blk
        # device layout: partition = msg-in-chunk, free = (block*chunk, dst)
        seg_dev = np.ascontiguousarray(
            seg.reshape(cfg.BPC * cpb, P, P).transpose(1, 0, 2)
        ).reshape(P, cfg.BPC * cpb * P).astype(bf16)
        xs = x_tab[c * cfg.SPC:(c + 1) * cfg.SPC]
        xs_dev = np.ascontiguousarray(
            xs.reshape(cfg.BPC, P, C).transpose(1, 0, 2)
        ).reshape(P, cfg.BPC * C)
        per_core.append({
            "seg": seg_dev,
            "idx_lo": wrap_calls(idx_lo[blocks].reshape(-1), cfg.CPC * P),
            "idx_hi": wrap_calls(idx_hi[blocks].reshape(-1), cfg.CPC * P),
            "x_self": xs_dev,
            "dinv_bc": np.ascontiguousarray(
                np.broadcast_to(dinv_slot[c * cfg.SPC:(c + 1) * cfg.SPC],
                                (P, cfg.SPC))).astype(np.float32),
        })
    return per_core, x_tab, node_to_slot, a_lo, a_hi


def _build_program(cfg, a_lo, a_hi, debug=False, skip_collective=False,
                   skip_gather=False, l2_table_xtab=False):
    cpb = a_lo + a_hi + 1
    nc = bacc.Bacc("TRN2", target_bir_lowering=False, debug=debug,
                   num_devices=cfg.NCORES)
    f32, b16, i16 = mybir.dt.float32, mybir.dt.bfloat16, mybir.dt.int16
    BPC, SPC, CPC, HALF, NPAD = cfg.BPC, cfg.SPC, cfg.CPC, cfg.HALF, cfg.NPAD

    x_tab = nc.dram_tensor("x_tab", [NPAD, C], b16, kind="ExternalInput")
    seg_in = nc.dram_tensor("seg", [P, BPC * cpb * P], b16, kind="ExternalInput")
    idx_lo_in = nc.dram_tensor("idx_lo", [P, BPC * a_lo * 8], i16,
                               kind="ExternalInput")
    idx_hi_in = nc.dram_tensor("idx_hi", [P, BPC * a_hi * 8], i16,
                               kind="ExternalInput")
    x_self_in = nc.dram_tensor("x_self", [P, BPC * C], b16, kind="ExternalInput")
    dinv_in = nc.dram_tensor("dinv_bc", [P, SPC], f32, kind="ExternalInput")
    w1_in = nc.dram_tensor("w1", [C, C], b16, kind="ExternalInput")
    w2_in = nc.dram_tensor("w2", [C, C], b16, kind="ExternalInput")
    b1_in = nc.dram_tensor("b1", [P, 1], f32, kind="ExternalInput")
    b2_in = nc.dram_tensor("b2", [P, 1], f32, kind="ExternalInput")
    out = nc.dram_tensor("out", [P, SPC], f32, kind="ExternalOutput")

    t2_shard = nc.dram_tensor("t2_shard", [SPC, C], b16)
    t2_full = nc.dram_tensor("t2_full", [NPAD, C], b16, addr_space="Shared")

    with tile.TileContext(nc) as tc:
        with (
            tc.tile_pool(name="const", bufs=1) as cpool,
            tc.tile_pool(name="msg", bufs=4) as mpool,
            tc.tile_pool(name="seg", bufs=3) as spool,
            tc.tile_pool(name="work", bufs=3) as wpool,
            tc.tile_pool(name="psum", bufs=2, space="PSUM") as ppool,
        ):
            idx_lo_sb = cpool.tile([P, BPC * a_lo * 8], i16)
            nc.sync.dma_start(idx_lo_sb[:], idx_lo_in[:])
            idx_hi_sb = cpool.tile([P, BPC * a_hi * 8], i16)
            nc.sync.dma_start(idx_hi_sb[:], idx_hi_in[:])
            dinv_sb = cpool.tile([P, SPC], f32)
            nc.sync.dma_start(dinv_sb[:], dinv_in[:])
            w_sb, bias_sb = [], []
            for w_i, b_i in ((w1_in, b1_in), (w2_in, b2_in)):
                w_t = cpool.tile([C, C], b16, tag=f"w{w_i.name}")
                nc.sync.dma_start(w_t[:], w_i[:])
                b_t = cpool.tile([P, 1], f32, tag=f"b{b_i.name}")
                nc.sync.dma_start(b_t[:], b_i[:])
                w_sb.append(w_t)
                bias_sb.append(b_t)
            ident = cpool.tile([P, P], b16)
            make_identity(nc, ident[:])
            t2_sb = cpool.tile([P, BPC, P], b16)   # layer-1 out, transposed
            xself_sb = cpool.tile([P, BPC * C], b16)
            nc.sync.dma_start(xself_sb[:], x_self_in[:])

            cc_inst = None
            for layer in range(2):
                tables = (x_tab, x_tab if l2_table_xtab else t2_full)[layer]
                gathers = []
                for half, a_c, idx_sb in ((0, a_lo, idx_lo_sb),
                                          (1, a_hi, idx_hi_sb)):
                    tab_ap = tables[:HALF, :] if half == 0 else tables[HALF:, :]
                    nch = BPC * a_c                 # total chunks this half
                    half_bufs = []
                    for k in range(-(-nch // CPC)):
                        ch = min(CPC, nch - k * CPC)
                        nidx = ch * P
                        mt = mpool.tile([P, ch, P], b16, tag=f"msg{half}")
                        if skip_gather:
                            nc.vector.memset(mt[:], 0.0)
                        else:
                            g = nc.gpsimd.dma_gather(
                                out_ap=mt[:],
                                in_ap=tab_ap,
                                idxs_ap=idx_sb[:, k * CPC * 8:
                                               k * CPC * 8 + nidx // 16],
                                num_idxs=nidx,
                                num_idxs_reg=nidx,
                                elem_size=C,
                            )
                            if layer == 1 and cc_inst is not None:
                                tile.add_dep_helper(
                                    g.ins, cc_inst.ins,
                                    reason="gather after allgather")
                        half_bufs.append(mt)
                    gathers.append(half_bufs)

                for bb in range(BPC):
                    seg_t = spool.tile([P, cpb, P], b16, tag="segt")
                    nc.sync.dma_start(
                        seg_t[:],
                        seg_in[:, bb * cpb * P:(bb + 1) * cpb * P].rearrange(
                            "p (t f) -> p t f", f=P),
                    )
                    if layer == 0:
                        self_ap = xself_sb[:, bb * C:(bb + 1) * C]
                    else:
                        self_ap = t2_sb[:, bb, :]

                    ppre = ppool.tile([P, P], f32, tag="ppre")
                    t = 0
                    for half, a_c in ((0, a_lo), (1, a_hi)):
                        for tt in range(a_c):
                            g = bb * a_c + tt
                            mt = gathers[half][g // CPC]
                            nc.tensor.matmul(
                                ppre[:],
                                lhsT=mt[:, g % CPC, :],
                                rhs=seg_t[:, t, :],
                                start=(t == 0), stop=False,
                            )
                            t += 1
                    nc.tensor.matmul(ppre[:], lhsT=self_ap,
                                     rhs=seg_t[:, t, :], start=False, stop=True)

                    pre_sb = wpool.tile([P, P], b16, tag="presb")
                    nc.vector.tensor_copy(pre_sb[:], ppre[:])
                    p2 = ppool.tile([P, P], f32, tag="p2")
                    nc.tensor.matmul(p2[:], lhsT=w_sb[layer][:], rhs=pre_sb[:],
                                     start=True, stop=True)
                    nc.vector.tensor_tensor(
                        out=p2[:], in0=p2[:],
                        in1=dinv_sb[:, bb * P:(bb + 1) * P],
                        op=mybir.AluOpType.mult,
                    )
                    if layer == 0:
                        o1 = wpool.tile([P, P], b16, tag="o1")
                        nc.scalar.activation(o1[:], p2[:],
                                             mybir.ActivationFunctionType.Relu,
                                             bias=bias_sb[0][:, :1])
                        pt2 = ppool.tile([P, P], b16, tag="pt2")
                        nc.tensor.transpose(pt2[:], o1[:], ident[:])
                        nc.vector.tensor_copy(t2_sb[:, bb, :], pt2[:])
                    else:
                        o2 = wpool.tile([P, P], f32, tag="o2")
                        nc.scalar.activation(o2[:], p2[:],
                                             mybir.ActivationFunctionType.Relu,
                                             bias=bias_sb[1][:, :1])
                        nc.sync.dma_start(out[:, bb * P:(bb + 1) * P], o2[:])

                if layer == 0:
                    nc.sync.dma_start(
                        t2_shard[:, :].rearrange("(b p) f -> p b f", p=P),
                        t2_sb[:],
                    )
                    if skip_collective:
                        nc.sync.dma_start(
                            t2_full[:SPC, :].rearrange("(b p) f -> p b f", p=P),
                            t2_sb[:],
                        )
                    else:
                        cc_inst = nc.gpsimd.collective_compute(
                            "AllGather",
                            mybir.AluOpType.bypass,
                            replica_groups=[list(range(cfg.NCORES))],
                            ins=[t2_shard[:, :].opt()],
                            outs=[t2_full[:, :].opt()],
                        )

    nc.compile()
    return nc


def make_in_maps(per_core, x_tab, W1, b1, W2, b2, cfg):
    W1 = np.asarray(W1, np.float32).astype(bf16)
    W2 = np.asarray(W2, np.float32).astype(bf16)
    b1c = np.ascontiguousarray(np.asarray(b1, np.float32).reshape(C, 1))
    b2c = np.ascontiguousarray(np.asarray(b2, np.float32).reshape(C, 1))
    in_maps = []
    for c in range(cfg.NCORES):
        pc = per_core[c]
        in_maps.append({
            "x_tab": x_tab, "seg": pc["seg"], "idx_lo": pc["idx_lo"],
            "idx_hi": pc["idx_hi"], "x_self": pc["x_self"],
            "dinv_bc": pc["dinv_bc"],
            "w1": W1, "w2": W2, "b1": b1c, "b2": b2c,
        })
    return in_maps


_CACHE = {}


def _get_program(cfg, a_lo, a_hi, **kw):
    key = (cfg.N, cfg.NCORES, cfg.BPC, a_lo, a_hi, tuple(sorted(kw.items())))
    if key not in _CACHE:
        _CACHE[key] = _build_program(cfg, a_lo, a_hi, **kw)
    return _CACHE[key]


def kernel(x, edge_index, W1, b1, W2, b2):
    cfg = CFG_FULL
    per_core, x_tab, node_to_slot, a_lo, a_hi = _preprocess(x, edge_index, cfg)
    in_maps = make_in_maps(per_core, x_tab, W1, b1, W2, b2, cfg)
    nc = _get_program(cfg, a_lo, a_hi)
    res = bass_utils.run_bass_kernel_spmd(nc, in_maps,
                                          core_ids=list(range(cfg.NCORES)))
    y_slot = np.concatenate([res.results[c]["out"] for c in range(cfg.NCORES)],
                            axis=1)
    return np.ascontiguousarray(y_slot[:, node_to_slot].T)

